# revision 81
# baseline (speedup 1.0000x reference)
"""MDTA (Restormer transposed attention) Trainium2 Bass kernel.

Strategy: data-parallel over batch (8 cores x 1 image each). Per core:

q,k path (attention contracts over 16384 spatial positions, so fp8
quantization noise washes out ~1/sqrt(N)):
  - qkv GEMM as ONE fp8e4 DoubleRow matmul per 512-col tile (K=192 via a
    [96,2,N] partition-interleaved resident fp8 copy of x).
  - depthwise 3x3 on PE as diagonal matmuls: 3 fp8 DoubleRow matmuls pair
    the (dy=-1,dx)/(dy=+1,dx) taps via an overlapping 3D AP (pair stride
    2W bytes, 16B-aligned), + 2 single taps; the 9th tap (0,-1) is fused
    into the PSUM->SBUF drain as a DVE scalar_tensor_tensor (free drain).
  - squared norms via DVE tensor_tensor_reduce; per-channel scale
    LAM*temp/||q|| folded into the PE transpose by streaming diag(scale)
    instead of identity; attention q_hat^T k_hat as fp8 DoubleRow matmuls
    over spatial block-pairs; softmax exp(x/LAM^2) absorbs the prescale.

v path (errors go straight to the output, so it stays bf16 end-to-end):
  - v GEMM in bf16 from a resident bf16 x copy (K=128 + K=64 matmuls).
  - depthwise as 9 bf16 single taps: (0,-1) fused in the DVE drain,
    V_DVE_TAPS more on DVE (bf16 2x STT), the rest diagonal PE matmuls.
  - y = M^T.T @ v_dw in bf16, where M^T = A^T w_out^T.
"""
import sys
sys.path.insert(0, "/opt/trn_rl_repo")
from contextlib import ExitStack

import numpy as np
import ml_dtypes

import concourse.bass as bass
import concourse.mybir as mybir
import concourse.tile as tile
from concourse import bacc
from concourse.bass_utils import run_bass_kernel_spmd

F32 = mybir.dt.float32
BF16 = mybir.dt.bfloat16
FP8 = mybir.dt.float8e4
DR = mybir.MatmulPerfMode.DoubleRow
AF = mybir.ActivationFunctionType
ALU = mybir.AluOpType
AX = mybir.AxisListType

C = 192
NHEADS = 8
HDIM = 24
H = W = 128
N = H * W            # 16384 spatial positions
NT = 512             # free-dim tile (4 image rows)
NTILES = N // NT     # 32
EPS = 1e-12
LAM = 128.0          # fp8 prescale for q_hat/k_hat; softmax divides by LAM^2
# output-channel chunks of the 576-row qkv: q = 0..191, k = 192..383,
# v = 384..575
CHUNKS = [(0, 128), (128, 128), (256, 128), (384, 128), (512, 64)]
# tap index ti = (dy+1)*3 + (dx+1)
TI = {(dy, dx): (dy + 1) * 3 + (dx + 1)
      for dy in (-1, 0, 1) for dx in (-1, 0, 1)}

NORM_GPSIMD = False   # squared-norm squares on gpsimd (else DVE TTR)
# v-path dwconv: taps beyond the fused (0,-1) drain tap to run on DVE
# (DVE STT runs at 1x, ~594ns/tile vs 213ns on PE -- only worth it when
# PE is the bottleneck and DVE idles; tune by measurement)
V_DVE_TAPS = []
V_PE_TAPS = [(0, 0), (0, 1), (-1, -1), (-1, 0), (-1, 1)] + \
    [t for t in ((1, -1), (1, 0), (1, 1)) if t not in V_DVE_TAPS]


def pair_ap(t2d, delta):
    """Insert an overlapping [delta, 2] dim at axis 1 (DoubleRow k-pair)."""
    ap = t2d.copy()
    ap.ap.insert(1, [delta, 2])
    return ap


def build_nc(reps=1, abl=()):  # noqa: C901
    nc = bacc.Bacc("TRN2", target_bir_lowering=False, debug=False)
    x_d = (nc.dram_tensor("x_scratch", [C, N], F32) if "dummyx" in abl
           else nc.dram_tensor("x", [C, N], F32, kind="ExternalInput"))
    wq_d = nc.dram_tensor("wq_dri", [96, 2 * 384], FP8, kind="ExternalInput")
    wv_d = nc.dram_tensor("wvT", [C, 192], BF16, kind="ExternalInput")
    wdd_d = nc.dram_tensor("w_dwd", [5, 128, 9 * 128], FP8, kind="ExternalInput")
    wdb_d = nc.dram_tensor("w_dwb", [2, 128, 9 * 128], BF16, kind="ExternalInput")
    wdn_d = nc.dram_tensor("w_dwn", [5, 128, 9], F32, kind="ExternalInput")  # negated
    wdp_d = nc.dram_tensor("w_dwp", [5, 128, 9], F32, kind="ExternalInput")  # taps
    wo_d = nc.dram_tensor("w_outT", [C, C], BF16, kind="ExternalInput")
    tmp_d = nc.dram_tensor("temp", [C, 1], F32, kind="ExternalInput")
    mask_d = nc.dram_tensor("mask", [C, C], BF16, kind="ExternalInput")
    eye_d = nc.dram_tensor("eye", [128, 128], BF16, kind="ExternalInput")
    if "dummyy" in abl:
        y_d = nc.dram_tensor("y_scratch", [C, N], F32)
        yprobe_d = nc.dram_tensor("y", [128, 4], F32, kind="ExternalOutput")
    else:
        y_d = nc.dram_tensor("y", [C, N], F32, kind="ExternalOutput")
        yprobe_d = None
    dbg = {}
    if "dbg" in abl:
        dbg["pre8"] = nc.dram_tensor("pre8_dbg", [3, 128, N], FP8,
                                     kind="ExternalOutput")
        dbg["preb"] = nc.dram_tensor("preb_dbg", [2, 128, N], BF16,
                                     kind="ExternalOutput")
        dbg["dw"] = nc.dram_tensor("dw_dbg", [5, 128, N], BF16,
                                   kind="ExternalOutput")
        dbg["a"] = nc.dram_tensor("a_dbg", [C, C], F32, kind="ExternalOutput")
        dbg["kt"] = nc.dram_tensor("kt_dbg", [128, 128 * C], FP8,
                                   kind="ExternalOutput")

    with tile.TileContext(nc) as tc, ExitStack() as ctx:
        wp = ctx.enter_context(tc.tile_pool(name="w", bufs=1))
        prep = ctx.enter_context(tc.tile_pool(name="pre", bufs=1))
        dwdp = ctx.enter_context(tc.tile_pool(name="dwd", bufs=1))
        sp = ctx.enter_context(tc.tile_pool(name="small", bufs=1))
        yp = ctx.enter_context(tc.tile_pool(name="y", bufs=2))
        gps = ctx.enter_context(tc.tile_pool(name="gps", bufs=2, space="PSUM"))
        dps = ctx.enter_context(tc.tile_pool(name="dps", bufs=2, space="PSUM"))

        # ---- persistent weights ----
        wq = wp.tile([96, 2 * 384], FP8, tag="wq")
        nc.sync.dma_start(wq[:], wq_d[:])
        wq3 = wq[:].rearrange("p (two m) -> p two m", two=2)
        wv = wp.tile([128, 192], BF16, tag="wv")
        wv1 = wp.tile([64, 192], BF16, tag="wv1")
        nc.sync.dma_start(wv[:], wv_d[0:128, :])
        nc.sync.dma_start(wv1[:], wv_d[128:192, :])
        wo0 = wp.tile([128, C], BF16, tag="wo0")
        wo1 = wp.tile([64, C], BF16, tag="wo1")
        nc.sync.dma_start(wo0[:], wo_d[0:128, :])
        nc.sync.dma_start(wo1[:], wo_d[128:192, :])
        # resident q,k depthwise diag weights (fp8) + tap scalars so
        # chunk pipelining isn't serialized on a shared weight buffer
        dwds = {}
        dwns = {}
        dwps = {}
        for ci in range(3):
            dwds[ci] = wp.tile([128, 9 * 128], FP8, tag=f"dwd{ci}",
                               name=f"dwd{ci}")
            nc.sync.dma_start(dwds[ci][:], wdd_d[ci])
            dwns[ci] = wp.tile([128, 9], F32, tag=f"dwn{ci}",
                               name=f"dwn{ci}")
            nc.sync.dma_start(dwns[ci][:], wdn_d[ci])
            dwps[ci] = wp.tile([128, 9], F32, tag=f"dwp{ci}",
                               name=f"dwp{ci}")
            nc.sync.dma_start(dwps[ci][:], wdp_d[ci])
        eye_s = wp.tile([128, 128], BF16, tag="eye")
        nc.sync.dma_start(eye_s[:], eye_d[:])
        tmp0 = wp.tile([128, 1], F32, tag="tmp0")
        tmp1 = wp.tile([64, 1], F32, tag="tmp1")
        nc.sync.dma_start(tmp0[:], tmp_d[0:128, :])
        nc.sync.dma_start(tmp1[:], tmp_d[128:192, :])
        mask0 = wp.tile([128, C], BF16, tag="mask0")
        mask1 = wp.tile([64, C], BF16, tag="mask1")
        nc.sync.dma_start(mask0[:], mask_d[0:128, :])
        nc.sync.dma_start(mask1[:], mask_d[128:192, :])
        # M^T bf16 (built per rep, used in phase 2); mt1 rows are
        # duplicated on partitions 64..127 to pair with v4 in xv[64:128]
        mt0 = wp.tile([128, C], BF16, tag="mt0")
        mt1 = wp.tile([128, C], BF16, tag="mt1")

        # resident copies of x: fp8 interleaved for q,k; bf16 for v.
        # xv shares one tile between xres1 (parts 0..63, x channels
        # 128..191) and the phase-2 v4 output (parts 64..127).
        xi = wp.tile([96, 2 * N], FP8, tag="xi")
        xres0 = wp.tile([128, N], BF16, tag="xres0")
        xv = wp.tile([128, N], BF16, tag="xv")
        with tc.tile_pool(name="xload", bufs=3) as xp:
            for t in range(NTILES):
                cols = slice(t * NT, (t + 1) * NT)
                xt0 = xp.tile([128, NT], F32, tag="x0")
                xt1 = xp.tile([64, NT], F32, tag="x1")
                nc.sync.dma_start(xt0[:], x_d[0:128, cols])
                nc.sync.dma_start(xt1[:], x_d[128:192, cols])
                nc.vector.tensor_copy(xi[0:96, cols], xt0[0:96, :])
                nc.vector.tensor_copy(
                    xi[0:32, N + t * NT:N + (t + 1) * NT], xt0[96:128, :])
                nc.vector.tensor_copy(
                    xi[32:64, N + t * NT:N + (t + 1) * NT], xt1[0:32, :])
                nc.vector.tensor_copy(
                    xi[64:96, N + t * NT:N + (t + 1) * NT], xt1[32:64, :])
                nc.scalar.copy(xres0[:, cols], xt0[:])
                nc.scalar.copy(xv[0:64, cols], xt1[:])

        def load_dwd(ci):
            if ci >= 3:
                dwd = dwdp.tile([128, 9 * 128], BF16, tag="dwb")
                nc.sync.dma_start(dwd[:], wdb_d[ci - 3])
            else:
                dwd = dwdp.tile([128, 9 * 128], FP8, tag="dwd")
                nc.sync.dma_start(dwd[:], wdd_d[ci])
            dwn = dwdp.tile([128, 9], F32, tag="dwn")
            nc.sync.dma_start(dwn[:], wdn_d[ci])
            dwpos = dwdp.tile([128, 9], F32, tag="dwpos")
            nc.sync.dma_start(dwpos[:], wdp_d[ci])
            return dwd, dwn, dwpos

        def dwconv_tile_qk(mw, pre, dwd, t):
            """PE taps for q,k tile t -> PSUM: center + (0,+1) singles and
            3 vertical fp8 DoubleRow pairs; (0,-1) left for the DVE drain."""
            base = t * NT
            if "nodrdw" in abl:  # fp8 singles fallback
                pd = dps.tile([mw, NT], F32, tag="d")
                taps = [(0, 0), (0, 1), (-1, -1), (-1, 0), (-1, 1),
                        (1, -1), (1, 0), (1, 1)]
                for i, (dy, dx) in enumerate(taps):
                    ti = TI[(dy, dx)]
                    sh = dy * W + dx
                    lo = max(0, -(base + sh))
                    hi = min(NT, N - base - sh)
                    nc.tensor.matmul(pd[:, lo:hi],
                                     dwd[0:mw, ti * 128:ti * 128 + mw],
                                     pre[0:mw, base + sh + lo:base + sh + hi],
                                     start=(i == 0), stop=(i == len(taps) - 1))
                return pd
            pd = dps.tile([mw, NT], F32, tag="d")
            ti = TI[(0, 0)]
            nc.tensor.matmul(pd[:], dwd[0:mw, ti * 128:ti * 128 + mw],
                             pre[0:mw, base:base + NT], start=True, stop=False)
            ti = TI[(0, 1)]
            hi = min(NT, N - base - 1)
            nc.tensor.matmul(pd[:, 0:hi], dwd[0:mw, ti * 128:ti * 128 + mw],
                             pre[0:mw, base + 1:base + 1 + hi],
                             start=False, stop=False)
            for j, dx in enumerate((-1, 0, 1)):
                ta, tb = TI[(-1, dx)], TI[(1, dx)]
                sa, sb = -W + dx, W + dx
                lo = max(0, -(base + sa))
                hi = min(NT, N - base - sb)
                last = (j == 2)
                lhsT = pair_ap(dwd[0:mw, ta * 128:ta * 128 + mw], 6 * 128)
                rhs = pair_ap(pre[0:mw, base + sa + lo:base + sa + hi], 2 * W)
                nc.tensor.matmul(pd[:, lo:hi], lhsT, rhs, start=False,
                                 stop=(last and lo == 0 and hi == NT),
                                 perf_mode=DR)
                if lo > 0:  # first tile: (dy=+1,dx) valid on [lob, lo)
                    lob = max(0, -(base + sb))
                    nc.tensor.matmul(pd[:, lob:lo],
                                     dwd[0:mw, tb * 128:tb * 128 + mw],
                                     pre[0:mw, base + sb + lob:base + sb + lo],
                                     start=False, stop=last)
                if hi < NT:  # last tile: (dy=-1,dx) valid on [hi, hia)
                    hia = min(NT, N - base - sa)
                    nc.tensor.matmul(pd[:, hi:hia],
                                     dwd[0:mw, ta * 128:ta * 128 + mw],
                                     pre[0:mw, base + sa + hi:base + sa + hia],
                                     start=False, stop=last)
            return pd

        def dwconv_tile_v(mw, pre, dwd, t, prow=0):
            """PE taps for v tile t -> PSUM: bf16 singles from V_PE_TAPS.
            prow: partition row offset (chunk 4 data lives on rows 64..127
            everywhere so walrus same-base-partition rules hold)."""
            base = t * NT
            pd = dps.tile([128, NT], F32, tag="d")
            rows = slice(prow, prow + mw)
            for i, (dy, dx) in enumerate(V_PE_TAPS):
                ti = TI[(dy, dx)]
                sh = dy * W + dx
                lo = max(0, -(base + sh))
                hi = min(NT, N - base - sh)
                nc.tensor.matmul(pd[rows, lo:hi],
                                 dwd[rows, ti * 128:ti * 128 + mw],
                                 pre[rows, base + sh + lo:base + sh + hi],
                                 start=(i == 0), stop=(i == len(V_PE_TAPS) - 1))
            return pd

        def drain_tap(mw, pre, dwpos, pd, t, dst, prow=0):
            """DVE drain: dst = pre[.,n-1]*w(0,-1) + psum (tap fused)."""
            base = t * NT
            lo = max(0, 1 - base)
            ti = TI[(0, -1)]
            rows = slice(prow, prow + mw)
            nc.vector.scalar_tensor_tensor(
                out=dst[rows, base + lo:base + NT],
                in0=pre[rows, base + lo - 1:base + NT - 1],
                scalar=dwpos[rows, ti:ti + 1], in1=pd[rows, lo:NT],
                op0=ALU.mult, op1=ALU.add)
            if lo > 0:
                nc.vector.tensor_copy(dst[rows, 0:lo], pd[rows, 0:lo])

        def edge_fixes(ci, pre, dwn, dst, prow=0):
            """Subtract row-wrap contamination of dx=+-1 taps (strided STT)."""
            mw = CHUNKS[ci][1]
            rows = slice(prow, prow + mw)
            for dy in (-1, 0, 1):
                ti_l = (dy + 1) * 3 + 0
                y0, y1 = max(0, 1 - dy), min(127, 128 - dy)
                out_ap = dst[rows, y0 * W:y1 * W + 1:W]
                src_ap = pre[rows, (y0 + dy) * W - 1:(y1 + dy) * W:W]
                nc.vector.scalar_tensor_tensor(
                    out=out_ap, in0=src_ap, scalar=dwn[rows, ti_l:ti_l + 1],
                    in1=out_ap, op0=ALU.mult, op1=ALU.add)
                ti_r = (dy + 1) * 3 + 2
                y0, y1 = max(0, -1 - dy), min(127, 126 - dy)
                out_ap = dst[rows, y0 * W + W - 1:y1 * W + W:W]
                src_ap = pre[rows, (y0 + dy + 1) * W:(y1 + dy + 1) * W + 1:W]
                nc.vector.scalar_tensor_tensor(
                    out=out_ap, in0=src_ap, scalar=dwn[rows, ti_r:ti_r + 1],
                    in1=out_ap, op0=ALU.mult, op1=ALU.add)

        # ================= per-rep body =================
        for _rep in range(reps):
         with tc.tile_pool(name="kT", bufs=1) as kTp, \
             tc.tile_pool(name="dwout", bufs=1) as dwp, \
             tc.tile_pool(name="junk", bufs=1) as jp, \
             tc.tile_pool(name="qt", bufs=3) as qtp, \
             tc.tile_pool(name="tps", bufs=2, space="PSUM") as tps, \
             tc.tile_pool(name="aps", bufs=2, space="PSUM") as aps:

            kT = kTp.tile([128, 128 * C], FP8, tag="kT")  # block b at cols b*C
            kT3 = kT[:].rearrange("p (blk c) -> p blk c", c=C)
            junkA = jp.tile([128, 512], BF16, tag="junkA")
            junkB = jp.tile([128, 512], BF16, tag="junkB")
            apq0 = aps.tile([128, C], F32, tag="attn")
            apq1 = aps.tile([64, C], F32, tag="attn")

            # staged bring-up level for hw bisection: 9 = everything
            qklvl = 9
            for a in abl:
                if a.startswith("qklvl"):
                    qklvl = int(a[5:])

            def qk_gemm(ci):
                """fp8 DoubleRow GEMM -> pre (fp8, aliases shared buffer)."""
                mc0, mw = CHUNKS[ci]
                preb = prep.tile([128, N], BF16, tag="preb")
                pre = preb[:].bitcast(FP8)[:, 0:N]
                for t in range(NTILES):
                    cols = slice(t * NT, (t + 1) * NT)
                    pg = gps.tile([mw, NT], F32, tag="g")
                    nc.tensor.matmul(pg[:], wq3[:, :, mc0:mc0 + mw],
                                     pair_ap(xi[0:96, cols], N),
                                     start=True, stop=True, perf_mode=DR)
                    nc.scalar.copy(pre[0:mw, cols], pg[:])
                return pre

            def qk_dw(ci, pre):
                """dwconv (DR pairs + singles + fused DVE drain tap)."""
                mc0, mw = CHUNKS[ci]
                dw = dwp.tile([128, N], BF16, tag="dw")
                for t in range(NTILES):
                    pd = dwconv_tile_qk(mw, pre, dwds[ci], t)
                    drain_tap(mw, pre, dwps[ci], pd, t, dw)
                edge_fixes(ci, pre, dwns[ci], dw)
                if dbg:
                    nc.sync.dma_start(dbg["pre8"][ci][0:mw], pre[0:mw, :])
                    nc.sync.dma_start(dbg["dw"][ci][0:mw], dw[0:mw, :])
                return dw

            def qk_norms(ci, dw, q_lo):
                """32 sq-norm slabs alternating ACT/DVE -> diag(scale)."""
                mw = CHUNKS[ci][1]
                stat = sp.tile([128, 32], F32, tag=f"stat{ci}")
                for s8 in range(32):
                    slab = dw[0:mw, s8 * 512:(s8 + 1) * 512]
                    if s8 % 2 == 0:
                        nc.scalar.activation(
                            junkA[0:mw, :], slab, AF.Square,
                            accum_out=stat[0:mw, s8:s8 + 1])
                    else:
                        nc.vector.tensor_tensor(junkB[0:mw, :], slab, slab,
                                                op=ALU.mult)
                        nc.vector.tensor_reduce(
                            stat[0:mw, s8:s8 + 1], junkB[0:mw, :],
                            axis=AX.X, op=ALU.add)
                n2 = sp.tile([128, 1], F32, tag=f"n2{ci}")
                nc.vector.tensor_reduce(n2[0:mw, :], stat[0:mw, 0:32],
                                        axis=AX.X, op=ALU.add)
                nc.scalar.activation(n2[0:mw, :], n2[0:mw, :], AF.Sqrt)
                nc.vector.tensor_scalar_max(n2[0:mw, :], n2[0:mw, :], EPS)
                sv = sp.tile([128, 1], F32, tag=f"sv{ci}")
                nc.vector.reciprocal(sv[0:mw, :], n2[0:mw, :])
                nc.vector.tensor_scalar_mul(sv[0:mw, :], sv[0:mw, :], LAM)
                if q_lo:  # leading q rows also get temperature
                    qw, toff = q_lo
                    nc.vector.tensor_tensor(
                        sv[0:qw, :], sv[0:qw, :],
                        (tmp0 if toff < 128 else tmp1)[toff % 128:toff % 128 + qw, :],
                        op=ALU.mult)
                diag = sp.tile([128, 128], BF16, tag=f"diag{ci}")
                nc.vector.tensor_scalar_mul(diag[0:mw, 0:mw],
                                            eye_s[0:mw, 0:mw], sv[0:mw, :])
                return diag

            def qk_attn(ci, dw, diag, k_cols, q_rows):
                """Scale-folded transposes (4/PSUM tile), k/q copies, and
                fp8 DR attention lagging 2 batches behind its copies."""
                mw = CHUNKS[ci][1]
                qtbs = {}

                def attn_batch(b0):
                    plo, phi, apsum, arow0 = q_rows
                    qw = phi - plo
                    qtb4 = qtbs.pop(b0)
                    for h in (0, 2):
                        nc.tensor.matmul(
                            apsum[arow0:arow0 + qw, :],
                            qtb4[:, h:h + 2, 0:qw],
                            kT3[:, b0 + h:b0 + h + 2, :],
                            start=(b0 + h == 0), stop=(b0 + h == 126),
                            perf_mode=DR, skip_group_check=True)

                for b0 in range(0, 128, 4):
                    pt = tps.tile([128, 4 * mw], F32, tag="t")
                    pt4 = pt[:].rearrange("p (four c) -> p four c", c=mw)
                    for h in range(4):
                        b = b0 + h
                        nc.tensor.matmul(
                            pt[:, h * mw:(h + 1) * mw],
                            dw[0:mw, b * 128:(b + 1) * 128],
                            diag[0:mw, 0:mw], start=True, stop=True)
                    if k_cols is not None:
                        plo, phi, koff = k_cols
                        nc.scalar.copy(
                            kT3[:, b0:b0 + 4, koff:koff + (phi - plo)],
                            pt4[:, 0:4, plo:phi])
                    if q_rows is not None:
                        plo, phi, apsum, arow0 = q_rows
                        qw = phi - plo
                        qtb = qtp.tile([128, 512], FP8, tag="qt")
                        qtb4 = qtb[:].rearrange("p (four c) -> p four c", c=128)
                        nc.vector.tensor_copy(qtb4[:, 0:4, 0:qw],
                                              pt4[:, 0:4, plo:phi])
                        qtbs[b0] = qtb4
                        if b0 >= 8:
                            attn_batch(b0 - 8)
                if q_rows is not None:
                    attn_batch(120)
                    attn_batch(124)

            # chunk-level software pipeline: emit the next chunk's GEMM
            # and dwconv before the previous chunk's norms/transposes so
            # PE always has work while ACT/DVE drain copies run.
            # k channels 256..383 (k-local 64..191); then q 128..191 + k
            # 192..255; then q 0..127
            if "vonly" in abl:
                nc.vector.memset(apq0[:], 0.0)
                nc.vector.memset(apq1[:], 0.0)
            else:
                pre2 = qk_gemm(2)
                dw2 = qk_dw(2, pre2)
                pre1 = qk_gemm(1)
                diag2 = qk_norms(2, dw2, None)
                qk_attn(2, dw2, diag2, (0, 128, 64), None)
                dw1 = qk_dw(1, pre1)
                pre0 = qk_gemm(0)
                diag1 = qk_norms(1, dw1, (64, 128))
                qk_attn(1, dw1, diag1, (64, 128, 0), (0, 64, apq1, 0))
                dw0 = qk_dw(0, pre0)
                diag0 = qk_norms(0, dw0, (128, 0))
                qk_attn(0, dw0, diag0, None, (0, 128, apq0, 0))

            # ---- attention: mask + softmax (logits are LAM^2 * true) ----
            def softmax_rows(apsum, msk, mw):
                a = (junkA if mw == 128 else junkB)[0:mw, 0:C]
                nc.vector.tensor_tensor(a, apsum[:], msk[0:mw, :], op=ALU.add)
                mx = sp.tile([128, 1], F32, tag="mx")
                nc.vector.tensor_reduce(mx[0:mw, :], a, axis=AX.X, op=ALU.max)
                nmx = sp.tile([128, 1], F32, tag="nmx")
                nc.vector.tensor_scalar_mul(nmx[0:mw, :], mx[0:mw, :],
                                            -1.0 / (LAM * LAM))
                nc.scalar.activation(a, a, AF.Exp, bias=nmx[0:mw, :],
                                     scale=1.0 / (LAM * LAM))
                sm = sp.tile([128, 1], F32, tag="sm")
                nc.vector.tensor_reduce(sm[0:mw, :], a, axis=AX.X, op=ALU.add)
                rsm = sp.tile([128, 1], F32, tag="rsm")
                nc.vector.reciprocal(rsm[0:mw, :], sm[0:mw, :])
                nc.vector.tensor_scalar_mul(a, a, rsm[0:mw, :])
                return a
            a0 = softmax_rows(apq0, mask0, 128)
            a1 = softmax_rows(apq1, mask1, 64)
            if dbg:
                nc.sync.dma_start(dbg["a"][0:128], a0[:])
                nc.sync.dma_start(dbg["a"][128:192], a1[:])
                nc.sync.dma_start(dbg["kt"][:], kT[:])

            # M^T[d,o] = sum_c A[c,d] w_outT[c,o]; K = c (192 -> 2 chunks)
            for dlo, dw_, mt in ((0, 128, mt0), (128, 64, mt1)):
                pm = tps.tile([128, C], F32, tag="t")
                nc.tensor.matmul(pm[0:dw_, :], a0[:, dlo:dlo + dw_],
                                 wo0[:], start=True, stop=False)
                nc.tensor.matmul(pm[0:dw_, :], a1[:, dlo:dlo + dw_],
                                 wo1[:], start=False, stop=True)
                nc.scalar.copy(mt[0:dw_, :], pm[0:dw_, :])
                if dw_ == 64:
                    nc.scalar.copy(mt[64:128, :], pm[0:64, :])

         # ---- phase 2: v chunks (bf16) -> v3 / xv[64:128], then y ----
         with tc.tile_pool(name="v3", bufs=1) as v3p:
            v3 = v3p.tile([128, N], BF16, tag="v3")
            if "qkonly" in abl:
                nc.vector.memset(v3[:], 0.0)
                nc.vector.memset(xv[64:128, :], 0.0)
            for ci, vt, prow in (() if "qkonly" in abl
                                 else ((3, v3, 0), (4, xv, 64))):
                mc0, mw = CHUNKS[ci]
                pre = prep.tile([128, N], BF16, tag="preb")
                dwd, dwn, dwpos = load_dwd(ci)
                for t in range(NTILES):
                    cols = slice(t * NT, (t + 1) * NT)
                    pg = gps.tile([mw, NT], F32, tag="g")
                    nc.tensor.matmul(pg[:], wv[:, mc0 - 384:mc0 - 384 + mw],
                                     xres0[:, cols], start=True, stop=False)
                    nc.tensor.matmul(pg[:], wv1[:, mc0 - 384:mc0 - 384 + mw],
                                     xv[0:64, cols], start=False, stop=True)
                    nc.scalar.copy(pre[prow:prow + mw, cols], pg[:])
                for t in range(NTILES):
                    pd = dwconv_tile_v(mw, pre, dwd, t, prow)
                    drain_tap(mw, pre, dwpos, pd, t, vt, prow)
                edge_fixes(ci, pre, dwn, vt, prow)
                if dbg:
                    nc.sync.dma_start(dbg["preb"][ci - 3][0:mw],
                                      pre[prow:prow + mw, :])
                    nc.sync.dma_start(dbg["dw"][ci][0:mw],
                                      vt[prow:prow + mw, :])

            for t in range(NTILES):
                cols = slice(t * NT, (t + 1) * NT)
                py0 = gps.tile([128, NT], F32, tag="g")
                nc.tensor.matmul(py0[:], mt0[:, 0:128], v3[:, cols],
                                 start=True, stop=False)
                nc.tensor.matmul(py0[:], mt1[64:128, 0:128], xv[64:128, cols],
                                 start=False, stop=True)
                y0 = yp.tile([128, NT], F32, tag="y0")
                nc.scalar.copy(y0[:], py0[:])
                nc.sync.dma_start(y_d[0:128, cols], y0[:])
                py1 = gps.tile([64, NT], F32, tag="g")
                nc.tensor.matmul(py1[:], mt0[:, 128:192], v3[:, cols],
                                 start=True, stop=False)
                nc.tensor.matmul(py1[:], mt1[64:128, 128:192], xv[64:128, cols],
                                 start=False, stop=True)
                y1 = yp.tile([64, NT], F32, tag="y1")
                nc.scalar.copy(y1[:], py1[:])
                nc.sync.dma_start(y_d[128:192, cols], y1[:])
            if yprobe_d is not None:
                nc.sync.dma_start(yprobe_d[:], y0[:, 0:4])

    nc.compile()
    return nc


def host_inputs(x, w_qkv, w_dw, w_out, temperature):
    """Host-side prep: per-core input maps."""
    FP8NP = ml_dtypes.float8_e4m3
    b = x.shape[0]
    wq = np.asarray(w_qkv, np.float32)          # [576, 192]
    # q,k rows in fp8 DR interleave: wq_dri[p, j, m] = w_qkv[m, j*96+p]
    wqkT = np.ascontiguousarray(wq[0:384].astype(FP8NP).astype(np.float32).T)
    wq_dri = np.stack([wqkT[0:96], wqkT[96:192]], axis=1).astype(FP8NP)
    # v rows bf16: wvT[c, m] = w_qkv[384+m, c]
    wvT = np.ascontiguousarray(wq[384:576].T).astype(ml_dtypes.bfloat16)

    w_dw9 = np.asarray(w_dw, np.float32).reshape(576, 9)
    # q,k PE taps run on fp8-quantized weights; the DVE drain tap
    # (0,-1)=ti 3 uses f32; v taps are bf16 (+ f32 for DVE taps).
    # edge_fixes must subtract exactly what was added.
    eff = w_dw9.astype(FP8NP).astype(np.float32)
    eff[:, 3] = w_dw9[:, 3]
    effv = w_dw9.astype(ml_dtypes.bfloat16).astype(np.float32)
    effv[:, 3] = w_dw9[:, 3]
    for dy, dx in V_DVE_TAPS:
        effv[:, (dy + 1) * 3 + (dx + 1)] = w_dw9[:, (dy + 1) * 3 + (dx + 1)]
    wdd = np.zeros((5, 128, 9 * 128), np.float32)
    wdb = np.zeros((2, 128, 9 * 128), np.float32)
    wdn = np.zeros((5, 128, 9), np.float32)
    wdp = np.zeros((5, 128, 9), np.float32)
    for ci, (s, wid) in enumerate(CHUNKS):
        r0 = 64 if ci == 4 else 0  # chunk 4 lives on partitions 64..127
        for t in range(9):
            d = wdd[ci] if ci < 3 else wdb[ci - 3]
            d[r0:r0 + wid, t * 128:t * 128 + wid][
                np.arange(wid), np.arange(wid)] = w_dw9[s:s + wid, t]
        wdn[ci, r0:r0 + wid, :] = -(eff if ci < 3 else effv)[s:s + wid, :]
        wdp[ci, r0:r0 + wid, :] = w_dw9[s:s + wid, :]
    temp_pc = np.repeat(np.asarray(temperature, np.float32).reshape(NHEADS), HDIM
                        ).reshape(C, 1)
    mask = np.full((C, C), -1e9, np.float32)
    for h in range(NHEADS):
        mask[h * HDIM:(h + 1) * HDIM, h * HDIM:(h + 1) * HDIM] = 0.0
    shared = {
        "wq_dri": wq_dri.reshape(96, 2 * 384),
        "wvT": wvT,
        "w_dwd": wdd.astype(FP8NP),
        "w_dwb": wdb.astype(ml_dtypes.bfloat16),
        "w_dwn": wdn,
        "w_dwp": wdp,
        "w_outT": np.ascontiguousarray(np.asarray(w_out, np.float32).T
                                       ).astype(ml_dtypes.bfloat16),
        "temp": temp_pc,
        "mask": mask.astype(ml_dtypes.bfloat16),
        "eye": np.eye(128, dtype=ml_dtypes.bfloat16),
    }
    return [dict(shared, x=np.ascontiguousarray(
        np.asarray(x[c], np.float32).reshape(C, N))) for c in range(b)]


_NC_CACHE = {}


def kernel(x, w_qkv, w_dw, w_out, temperature):
    x = np.asarray(x)
    if "nc" not in _NC_CACHE:
        _NC_CACHE["nc"] = build_nc()
    nc = _NC_CACHE["nc"]
    in_maps = host_inputs(x, w_qkv, w_dw, w_out, temperature)
    res = run_bass_kernel_spmd(nc, in_maps, list(range(8)))
    out = np.stack([res.results[c]["y"].reshape(C, H, W) for c in range(8)])
    return out.astype(np.float32)
